# revision 35
# baseline (speedup 1.0000x reference)
"""3-layer GAT on 8 Trainium2 NeuronCores (Bass/Tile).

Strategy (dst-node graph partition, per sharding hint):
  - Each core owns a contiguous slice of 6250 dst nodes and all edges into them.
  - Per layer: data-parallel projection of the local node slice with an
    extended weight [W | W@al | W@ar] producing packed table rows
    [feat | ex-slot | el] (bf16) plus a local er table. The table is
    replicated with TWO AllGathers (local blocks [0,24) then [24,49)) so each
    half is its own tensor (dma_gather needs zero-base tensors and int16 row
    ids) and the collectives overlap projection/edge work.
  - Edge phase: per-edge rows fetched with dma_gather in 1024-row chunks
    rotated over the 4 SWDGE queues (small chunks overlap; 4096-row ones
    serialize). The chunk loop is software-pipelined: gathers + one-hot
    streams issue LA=3 chunks ahead of the vector/agg stages, er-expand
    matmuls one stage behind the loads, so PE's in-order queue never gates
    the next gather. oh streams on the sync HWDGE ring, ohT on the scalar
    ring. Attention uses exp without max-subtraction (softmax is
    shift-invariant; |e| <= ~2 here); per-128-edge-tile one-hot matmuls
    accumulate ex-weighted feature sums + softmax denominators into PSUM per
    128-node dst block; epilogue divides, adds bias, applies ELU and
    PE-transposes into the next layer's lhsT layout.
"""
import sys

import numpy as np
import ml_dtypes

try:
    from concourse import bass, mybir, tile, bacc  # noqa: F401
except ImportError:  # pragma: no cover
    sys.path.insert(0, "/opt/trn_rl_repo")
    from concourse import bass, mybir, tile, bacc  # noqa: F401
from concourse.bass_utils import run_bass_kernel_spmd

bf16 = ml_dtypes.bfloat16
f32 = np.float32

N = 50000
E = 800000
NEG = 0.2
NCORES = 8
NLOC = N // NCORES            # 6250
BLK = 128
NBLK = 49                     # ceil(6250/128)
NLOC_PAD = NBLK * BLK         # 6272
TILE = 128
BA = 24                       # table half A: local blocks [0, 24)
RA = BA * BLK                 # 3072 rows per core, 8*3072=24576 total
RB = NLOC_PAD - RA            # 3200 rows per core, 8*3200=25600 total
CH = 8                        # tiles per gather chunk (1024 rows: small
                              # enough that gathers on the 4 SWDGE queues
                              # overlap ~2.7x instead of serializing)
LA = 3                        # chunk software-pipeline depth

import os
DEBUG_PHASE = os.environ.get("KGAT_DEBUG", "")  # "", "proj", "gath", "nomm"

# layer configs: (in_ktiles, H, D, ROW, EXO, ELO, rhsN)
LAYERS = [
    dict(kt=2, H=4, D=32, HD=128, ROW=256, EXO=128, ELO=132, rhsN=132),
    dict(kt=1, H=4, D=32, HD=128, ROW=256, EXO=128, ELO=132, rhsN=132),
    dict(kt=1, H=1, D=64, HD=64, ROW=128, EXO=64, ELO=65, rhsN=65),
]


def _wrap_idx(vals):
    """int16 gather-index layout: element i at [i%16, i//16], replicated to
    all 8 groups of 16 partitions."""
    n = len(vals)
    assert n % 16 == 0
    arr = np.asarray(vals, np.int16).reshape(-1, 16).T  # [16, n//16]
    return np.tile(arr, (8, 1))


def _structure(src, dst):
    """Shared tile schedule + per-core index/one-hot arrays."""
    counts = np.zeros((NCORES, NBLK, 2), np.int64)
    per_core = []
    for k in range(NCORES):
        lo = k * NLOC
        m = (dst >= lo) & (dst < lo + NLOC)
        eidx = np.nonzero(m)[0]
        d_loc = dst[eidx] - lo
        half = ((src[eidx] % NLOC) >= RA).astype(np.int64)
        blk = d_loc // BLK
        order = np.lexsort((d_loc, blk, half))
        eidx, d_loc, half, blk = (a[order] for a in (eidx, d_loc, half, blk))
        per_core.append((eidx, d_loc, half, blk))
        np.add.at(counts[k], (blk, half), 1)
    T = np.maximum(np.ceil(counts / TILE).astype(np.int64).max(axis=0), 1)

    # shared schedule: half-major, block order; tiles per (b, h) = T[b, h]
    tile_block, tile_start, tile_stop, tile_half = [], [], [], []
    for h in range(2):
        for b in range(NBLK):
            for t in range(T[b, h]):
                tile_block.append(b)
                tile_half.append(h)
                tile_start.append(t == 0)
                tile_stop.append(t == T[b, h] - 1)
    S = len(tile_block)
    S_A = int(T[:, 0].sum())

    cores = []
    for k in range(NCORES):
        eidx, d_loc, half, blk = per_core[k]
        src_rows = np.zeros(S * TILE, np.int64)   # half-table row per slot
        oh = np.zeros((128, S * TILE), bf16)
        pos = 0
        for h in range(2):
            for b in range(NBLK):
                sel = np.nonzero((blk == b) & (half == h))[0]
                ns = len(sel)
                sl = slice(pos, pos + ns)
                s_glob = src[eidx[sel]]
                c_own = s_glob // NLOC
                l_own = s_glob % NLOC
                if h == 0:
                    src_rows[sl] = c_own * RA + l_own
                else:
                    src_rows[sl] = c_own * RB + (l_own - RA)
                slots = pos + np.arange(ns)
                oh[slots % 128, (slots // 128) * 128 +
                   (d_loc[sel] - b * BLK)] = 1.0
                pos += T[b, h] * TILE
        assert src_rows.max() < 32768 and src_rows.min() >= 0
        ohT = np.ascontiguousarray(
            oh.reshape(128, S, TILE).transpose(2, 1, 0)).reshape(
                128, S * TILE)
        cores.append(dict(
            idx_src=_wrap_idx(src_rows),
            oh=oh,
            ohT=ohT,
        ))
    meta = dict(T=T, S=S, S_A=S_A,
                tile_block=tile_block, tile_start=tile_start,
                tile_stop=tile_stop)
    return meta, cores


def _chunks(t0, t1):
    out = []
    t = t0
    while t < t1:
        c = min(CH, t1 - t)
        out.append((t, c))
        t += c
    return out


def _build_program(meta):
    from concourse.masks import make_identity
    dt = mybir.dt
    S, S_A = meta["S"], meta["S_A"]
    tb, tst, tsp = meta["tile_block"], meta["tile_start"], meta["tile_stop"]

    nc = bacc.Bacc("TRN2", target_bir_lowering=False, debug=False,
                   num_devices=NCORES, num_swdge_queues=4)
    xT_in = nc.dram_tensor("xT", [128, 2 * NLOC_PAD], dt.bfloat16,
                           kind="ExternalInput")
    w_in = [nc.dram_tensor(f"W{i+1}", [128, LAYERS[i]["kt"] * (
        LAYERS[i]["HD"] + 2 * LAYERS[i]["H"])], dt.bfloat16,
        kind="ExternalInput") for i in range(3)]
    b_in = [nc.dram_tensor(f"b{i+1}", [128, LAYERS[i]["HD"]], dt.float32,
                           kind="ExternalInput") for i in range(3)]
    isrc_in = nc.dram_tensor("idx_src", [128, S * 8], dt.int16,
                             kind="ExternalInput")
    oh_in = nc.dram_tensor("oh", [128, S * TILE], dt.bfloat16,
                           kind="ExternalInput")
    ohT_in = nc.dram_tensor("ohT", [128, S * TILE], dt.bfloat16,
                            kind="ExternalInput")
    out_ext = nc.dram_tensor("out", [NLOC_PAD, 64], dt.float32,
                             kind="ExternalOutput")

    with tile.TileContext(nc) as tc:
        with (
            tc.tile_pool(name="const", bufs=1) as constp,
            tc.tile_pool(name="acts", bufs=1) as actsp,
            tc.tile_pool(name="stage", bufs=1) as stagep,
            tc.tile_pool(name="ers", bufs=1) as ersp,
            tc.tile_pool(name="stream", bufs=8) as streamp,
            tc.tile_pool(name="epi", bufs=2) as epip,
            tc.tile_pool(name="psA", bufs=2, space="PSUM") as psA,
            tc.tile_pool(name="psB", bufs=2, space="PSUM") as psB,
            tc.tile_pool(name="dram", bufs=1, space="DRAM") as dram,
        ):
            ident = constp.tile([128, 128], dt.bfloat16, tag="ident")
            make_identity(nc, ident[:])
            isrc_sb = constp.tile([128, S * 8], dt.int16, tag="isrc")
            nc.sync.dma_start(out=isrc_sb[:], in_=isrc_in[:])
            w_sb, b_sb = [], []
            for i, cfg in enumerate(LAYERS):
                nw = cfg["HD"] + 2 * cfg["H"]
                w = constp.tile([128, cfg["kt"], nw], dt.bfloat16,
                                tag=f"w{i}")
                nc.sync.dma_start(out=w[:], in_=w_in[i][:].rearrange(
                    "p (k c) -> p k c", k=cfg["kt"]))
                w_sb.append(w)
                bb = constp.tile([128, cfg["HD"]], dt.float32, tag=f"b{i}")
                nc.sync.dma_start(out=bb[:], in_=b_in[i][:])
                b_sb.append(bb)

            xT = actsp.tile([128, 2, NLOC_PAD], dt.bfloat16, tag="acts")
            nc.sync.dma_start(out=xT[:], in_=xT_in[:].rearrange(
                "p (k c) -> p k c", k=2))

            hT_prev = xT  # [128, kt, NLOC_PAD] layout; kt collapses via view
            for li, cfg in enumerate(LAYERS):
                H, D, HD = cfg["H"], cfg["D"], cfg["HD"]
                ROW, EXO, ELO, rhsN = (cfg[x] for x in
                                       ("ROW", "EXO", "ELO", "rhsN"))
                kt = cfg["kt"]
                last = li == 2

                tblA_loc = dram.tile([RA, ROW], dt.bfloat16, tag=f"tla{li}")
                tblB_loc = dram.tile([RB, ROW], dt.bfloat16, tag=f"tlb{li}")
                tblA_full = dram.tile([NCORES * RA, ROW], dt.bfloat16,
                                      tag=f"tfa{li}")
                tblB_full = dram.tile([NCORES * RB, ROW], dt.bfloat16,
                                      tag=f"tfb{li}")

                # ---- projection: table rows + er table ----
                # half A (blocks [0,BA)) is DMAd + AllGathered as soon as its
                # blocks are projected so the collective overlaps the rest.
                tbl_sb = stagep.tile([128, NBLK, ROW], dt.bfloat16,
                                     tag="stage")
                er_sb = ersp.tile([128, NBLK, H], dt.bfloat16, tag="ers")
                nc.vector.memset(tbl_sb[:], 0.0)

                def proj_block(b):
                    pp = psB.tile([128, HD + 2 * H], dt.float32, tag="proj",
                                  space="PSUM")
                    for k in range(kt):
                        if li == 0:
                            lhsT = hT_prev[:, k, b * BLK:(b + 1) * BLK]
                        else:
                            lhsT = hT_prev[:, b * BLK:(b + 1) * BLK]
                        nc.tensor.matmul(pp[:], lhsT=lhsT,
                                         rhs=w_sb[li][:, k, :],
                                         start=(k == 0), stop=(k == kt - 1))
                    nc.vector.tensor_copy(out=tbl_sb[:, b, 0:HD],
                                          in_=pp[:, 0:HD])
                    nc.vector.tensor_copy(out=tbl_sb[:, b, ELO:ELO + H],
                                          in_=pp[:, HD:HD + H])
                    nc.vector.tensor_copy(out=er_sb[:, b, 0:H],
                                          in_=pp[:, HD + H:HD + 2 * H])

                for b in range(BA):
                    proj_block(b)
                nc.sync.dma_start(
                    out=tblA_loc[:].rearrange("(b p) c -> p b c", p=128),
                    in_=tbl_sb[:, 0:BA, :])
                nc.gpsimd.collective_compute(
                    "AllGather", mybir.AluOpType.bypass,
                    replica_groups=[list(range(NCORES))],
                    ins=[tblA_loc[:].opt()], outs=[tblA_full[:].opt()])
                for b in range(BA, NBLK):
                    proj_block(b)
                nc.sync.dma_start(
                    out=tblB_loc[:].rearrange("(b p) c -> p b c", p=128),
                    in_=tbl_sb[:, BA:NBLK, :])
                nc.gpsimd.collective_compute(
                    "AllGather", mybir.AluOpType.bypass,
                    replica_groups=[list(range(NCORES))],
                    ins=[tblB_loc[:].opt()], outs=[tblB_full[:].opt()])

                # ---- edge phase ----
                accA = stagep.tile([128, NBLK, rhsN], dt.float32, tag="stage")
                if last:
                    outacc = stagep.tile([128, NBLK, rhsN], dt.float32,
                                         tag="outacc")

                hT_new = None
                if not last:
                    hT_new = actsp.tile([128, NLOC_PAD], dt.bfloat16,
                                        tag="acts")

                cur = {"psum": None, "b": None, "half": None}
                chunk_no = [0]

                def finish_block(cur=cur, li=li, H=H, D=D, HD=HD, rhsN=rhsN,
                                 accA=accA, hT_new=hT_new, last=last):
                    ps, b, half = cur["psum"], cur["b"], cur["half"]
                    if ps is None:
                        return
                    if half == 0:
                        nc.vector.tensor_copy(out=accA[:, b, :], in_=ps[:])
                        return
                    sm = epip.tile([128, rhsN], mybir.dt.float32, tag="sm")
                    nc.vector.tensor_tensor(out=sm[:], in0=ps[:],
                                            in1=accA[:, b, :],
                                            op=mybir.AluOpType.add)
                    dr = epip.tile([128, H], mybir.dt.float32, tag="dr")
                    nc.vector.tensor_scalar_add(out=dr[:],
                                                in0=sm[:, HD:HD + H],
                                                scalar1=1e-9)
                    nc.vector.reciprocal(out=dr[:], in_=dr[:])
                    q = epip.tile([128, HD], mybir.dt.float32, tag="q")
                    nc.vector.tensor_tensor(
                        out=q[:].rearrange("p (h d) -> p h d", h=H),
                        in0=sm[:, 0:HD].rearrange("p (h d) -> p h d", h=H),
                        in1=dr[:].rearrange("p (h o) -> p h o", h=H)
                            .to_broadcast([128, H, D]),
                        op=mybir.AluOpType.mult)
                    # + bias (host-replicated to all 128 partitions)
                    nc.vector.tensor_tensor(
                        out=q[:], in0=q[:], in1=b_sb[li][:],
                        op=mybir.AluOpType.add)
                    if last:
                        nc.vector.tensor_copy(out=outacc[:, b, 0:HD],
                                              in_=q[:])
                        return
                    # elu: relu(q) + exp(min(q,0)) - 1
                    m = epip.tile([128, HD], mybir.dt.float32, tag="m")
                    nc.vector.tensor_scalar_min(out=m[:], in0=q[:],
                                                scalar1=0.0)
                    nc.scalar.activation(m[:], m[:],
                                         mybir.ActivationFunctionType.Exp)
                    hb = epip.tile([128, HD], mybir.dt.float32, tag="hb")
                    nc.vector.scalar_tensor_tensor(
                        out=hb[:], in0=q[:], scalar=0.0, in1=m[:],
                        op0=mybir.AluOpType.max, op1=mybir.AluOpType.add)
                    hbb = epip.tile([128, HD], mybir.dt.bfloat16, tag="hbb")
                    nc.vector.tensor_scalar_add(out=hbb[:], in0=hb[:],
                                                scalar1=-1.0)
                    tp = psA.tile([128, 128], mybir.dt.bfloat16, tag="tp",
                                  space="PSUM")
                    nc.tensor.transpose(tp[:], hbb[:], ident[:])
                    nc.vector.tensor_copy(
                        out=hT_new[:, b * BLK:(b + 1) * BLK], in_=tp[:])

                # software-pipelined chunk stream: S0 (gather + one-hot
                # loads) runs LA chunks ahead of S2 (vector chain + agg
                # matmuls); S1 (er-expand matmuls) runs one stage behind S0
                # so PE's in-order queue never gates the next gather.
                chunks = ([(0, c0, cn) for (c0, cn) in _chunks(0, S_A)] +
                          [(1, c0, cn) for (c0, cn) in _chunks(S_A, S)])
                ncks = len(chunks)
                st = {}

                def s0(i):
                    hf, c0, cn = chunks[i]
                    tblh = tblA_full[:] if hf == 0 else tblB_full[:]
                    ni = cn * TILE
                    gath = streamp.tile([128, CH, ROW], mybir.dt.bfloat16,
                                        tag="gath")
                    nc.gpsimd.dma_gather(
                        out_ap=gath[:, 0:cn, :], in_ap=tblh,
                        idxs_ap=isrc_sb[:, c0 * 8:c0 * 8 + cn * 8],
                        num_idxs=ni, num_idxs_reg=ni, elem_size=ROW,
                        single_packet=False, queue_num=i % 4)
                    ohb = streamp.tile([128, CH * TILE],
                                       mybir.dt.bfloat16, tag="oh")
                    nc.sync.dma_start(
                        out=ohb[:, 0:cn * TILE],
                        in_=oh_in[:, c0 * TILE:(c0 + cn) * TILE])
                    ohTb = streamp.tile([128, CH * TILE],
                                        mybir.dt.bfloat16, tag="ohT")
                    nc.scalar.dma_start(
                        out=ohTb[:, 0:cn * TILE],
                        in_=ohT_in[:, c0 * TILE:(c0 + cn) * TILE])
                    st[i] = dict(gath=gath, ohb=ohb, ohTb=ohTb)

                def s1(i):
                    hf, c0, cn = chunks[i]
                    d = st[i]
                    per = psB.tile([128, CH * H], mybir.dt.float32,
                                   tag="er", name="erps", space="PSUM")
                    for t in range(cn):
                        nc.tensor.matmul(
                            per[:, t * H:(t + 1) * H],
                            lhsT=d["ohTb"][:, t * TILE:(t + 1) * TILE],
                            rhs=er_sb[:, tb[c0 + t], 0:H],
                            start=True, stop=True)
                    d["per"] = per

                def s2(i):
                    hf, c0, cn = chunks[i]
                    d = st.pop(i)
                    gath, ohb, per = d["gath"], d["ohb"], d["per"]
                    est = streamp.tile([128, CH, H], mybir.dt.float32,
                                       tag="est")
                    nc.vector.tensor_tensor(
                        out=est[:, 0:cn, :],
                        in0=gath[:, 0:cn, ELO:ELO + H],
                        in1=per[:, 0:cn * H].rearrange(
                            "p (c h) -> p c h", h=H),
                        op=mybir.AluOpType.add)
                    nc.vector.scalar_tensor_tensor(
                        out=est[:, 0:cn, :], in0=est[:, 0:cn, :],
                        scalar=NEG, in1=est[:, 0:cn, :],
                        op0=mybir.AluOpType.mult,
                        op1=mybir.AluOpType.max)
                    nc.scalar.activation(
                        gath[:, 0:cn, EXO:EXO + H], est[:, 0:cn, :],
                        mybir.ActivationFunctionType.Exp)
                    nc.vector.tensor_tensor(
                        out=gath[:, 0:cn, 0:HD].rearrange(
                            "p c (h d) -> p c h d", h=H),
                        in0=gath[:, 0:cn, 0:HD].rearrange(
                            "p c (h d) -> p c h d", h=H),
                        in1=gath[:, 0:cn, EXO:EXO + H]
                            .rearrange("p c (h o) -> p c h o", h=H)
                            .to_broadcast([128, cn, H, D]),
                        op=mybir.AluOpType.mult)
                    for t in range(cn):
                        g = c0 + t
                        if tst[g]:
                            finish_block()
                            cur["psum"] = psA.tile([128, rhsN],
                                                   mybir.dt.float32,
                                                   tag="agg", name="aggp",
                                                   space="PSUM")
                            cur["b"], cur["half"] = tb[g], hf
                        nc.tensor.matmul(
                            cur["psum"][:],
                            lhsT=ohb[:, t * TILE:(t + 1) * TILE],
                            rhs=gath[:, t, 0:rhsN],
                            start=tst[g], stop=tsp[g])

                for i in range(ncks + LA):
                    if i < ncks:
                        s0(i)
                    if LA - 1 <= i < ncks + LA - 1:
                        s1(i - (LA - 1))
                    if i >= LA:
                        s2(i - LA)
                finish_block()
                cur["psum"] = None

                if last:
                    nc.sync.dma_start(
                        out=out_ext[:].rearrange("(b p) c -> p b c", p=128),
                        in_=outacc[:, :, 0:64])
                else:
                    hT_prev = hT_new
    nc.finalize()
    return nc


def kernel(**inputs):
    x = np.asarray(inputs["x"], f32)
    src = np.asarray(inputs["src"]).astype(np.int64)
    dst = np.asarray(inputs["dst"]).astype(np.int64)

    meta, cores = _structure(src, dst)

    # host weight prep: Wext = [W | W@al_h | W@ar_h] per layer
    def wext(W, al, ar):
        W = np.asarray(W, f32)
        al = np.asarray(al, f32)
        ar = np.asarray(ar, f32)
        Hh, Dd = al.shape
        Wl = np.stack([W[:, h * Dd:(h + 1) * Dd] @ al[h] for h in range(Hh)], 1)
        Wr = np.stack([W[:, h * Dd:(h + 1) * Dd] @ ar[h] for h in range(Hh)], 1)
        return np.concatenate([W, Wl, Wr], axis=1)  # [in, HD+2H]

    wx = [wext(inputs["W1"], inputs["al1"], inputs["ar1"]),
          wext(inputs["W2"], inputs["al2"], inputs["ar2"]),
          wext(inputs["W3"], inputs["al3"], inputs["ar3"])]
    w_arrs = []
    for i, cfg in enumerate(LAYERS):
        kt, nw = cfg["kt"], cfg["HD"] + 2 * cfg["H"]
        a = np.zeros((128, kt, nw), bf16)
        for k in range(kt):
            a[:, k, :] = wx[i][k * 128:(k + 1) * 128, :].astype(bf16)
        w_arrs.append(a.reshape(128, kt * nw))
    b_arrs = [np.tile(np.asarray(inputs[f"b{i+1}"], f32).reshape(1, -1),
                      (128, 1)) for i in range(3)]

    nc = _build_program(meta)

    in_maps = []
    for k in range(NCORES):
        lo = k * NLOC
        xT = np.zeros((128, 2, NLOC_PAD), bf16)
        xs = x[lo:lo + NLOC].astype(bf16)     # [6250, 256]
        for kk in range(2):
            xT[:, kk, 0:NLOC] = xs[:, kk * 128:(kk + 1) * 128].T
        in_maps.append({
            "xT": xT.reshape(128, 2 * NLOC_PAD),
            "W1": w_arrs[0], "W2": w_arrs[1], "W3": w_arrs[2],
            "b1": b_arrs[0], "b2": b_arrs[1], "b3": b_arrs[2],
            "idx_src": cores[k]["idx_src"],
            "oh": cores[k]["oh"],
            "ohT": cores[k]["ohT"],
        })

    trace = bool(os.environ.get("KGAT_TRACE"))
    res = run_bass_kernel_spmd(nc, in_maps, core_ids=list(range(NCORES)),
                               trace=trace)
    global LAST_RESULTS
    LAST_RESULTS = res
    out = np.concatenate([res.results[k]["out"][:NLOC]
                          for k in range(NCORES)], axis=0)
    return out.astype(f32)


LAST_RESULTS = None


if __name__ == "__main__":
    import jax
    sys.path.insert(0, "/root/problem")
    import reference as ref
    with jax.default_device(jax.devices("cpu")[0]):
        inp = {k: np.asarray(v) for k, v in ref.setup_inputs().items()}
        expected = np.asarray(ref.reference(**inp))
    got = kernel(**inp)
    err = np.abs(got - expected).max()
    rel = err / np.abs(expected).max()
    print(f"abs err {err:.6f}  rel(absmax) {rel:.6f}")

